# revision 28
# baseline (speedup 1.0000x reference)
"""Trainium2 Bass kernel for EnhancedPathAwareECA.

Data-parallel over batch: 16 examples split as 2 per NeuronCore across 8 cores
(no collectives — per-example stats are local). Each core streams its slice of
x through SBUF exactly once: load -> per-path sum over l -> tiny
attention/LN/MLP chain -> in-place channel scaling -> store.

fp16 I/O: x is downcast to fp16 on the host before upload and the output is
stored fp16 and upcast on the host — halves HBM traffic vs f32 (the f32
schedule was already at the chip HBM roofline, ~197 us). All pooled sums
accumulate in f32 (ACT accum_out / DVE reduce output dtype), the stats chain
is f32, and only the streamed tiles + final multiply are fp16 (x quantization
2^-11 -> rel err ~3e-4, far under the 2e-2 gate).

Schedule notes (hard-won on HW):
- Loads own the sync HWDGE ring exclusively; stores ride the scalar (ACT)
  HWDGE ring. HWDGE rings are FIFO: a store waiting on its scale multiply
  would head-of-line-block every later load if they shared a ring. The last
  example's h0 stores switch to the then-idle sync ring (dual-ring drain).
- Each 2 MiB path tile is two independent 1 MiB half-tiles in a 24-slot pool:
  8 spare slots let the next example prefetch through the stats seam.
- Per-path sums: h0 on ACT (activation Copy + accum_out), h1 on DVE
  (reduce_sum) — both are 1x-rate ops, and splitting engines keeps DVE from
  backlogging so the seam-critical last reduce dequeues immediately.
- Scale multiplies on DVE: fp32 tensor_scalar is 2x-rate (2x_2P mode);
  ACT Copy is ~1.7x slower and would pace the drain.
- Stats chain avoids ACT table swaps where possible: everything stays in the
  'sigmoid_and_others' set (sigmoid + erf-based exact gelu + square via DVE);
  only Sqrt forces 2 swaps/example (DVE pow is rejected by walrus, no DVE
  sqrt exists), mostly hidden under concurrent PE/DVE chain ops.
- All weight-only folds (combined 9-tap conv kernel = combine_w-mixed conv1/
  conv2 taps with the 1/l mean fold, 1/D LayerNorm fold into ln_g, b1/sqrt(2)
  for the erf gelu) are precomputed on the host.
"""

import sys
from contextlib import ExitStack

import numpy as np

sys.path.insert(0, "/opt/trn_rl_repo")

N_CORES = 8
B, C, L = 16, 1024, 4096
P, D = 8, 128            # paths, dims per path (C = P*D)
BLOC = B // N_CORES      # examples per core
LN_EPS = 1e-5
XBUFS = 16               # 1 MiB fp16 full-row tile slots (16 MiB SBUF) — the
                         # whole 2-example core slice fits; loads never stall
                         # on pool recycling
RSQRT_POW = False        # DVE pow is rejected by walrus (tensor_scalar_valid_ops)
USE_TTR = False          # tensor_tensor_reduce sums: NRT_EXEC_UNIT_UNRECOVERABLE
                         # status_code=101 on HW (in0==in1==out aliasing?)

_cached = None


def _build():
    import concourse.tile as tile
    from concourse import bacc, masks, mybir

    f32 = mybir.dt.float32
    f16 = mybir.dt.float16
    AX = mybir.AxisListType
    OP = mybir.AluOpType
    AF = mybir.ActivationFunctionType

    nc = bacc.Bacc(
        "TRN2",
        target_bir_lowering=False,
        debug=False,
        num_devices=N_CORES,
    )

    x_in = nc.dram_tensor("x_local", [BLOC, C, L], f16, kind="ExternalInput")
    a9_d = nc.dram_tensor("a9", [P, 9], f32, kind="ExternalInput")
    cb_d = nc.dram_tensor("cb8", [P, 1], f32, kind="ExternalInput")
    lng_d = nc.dram_tensor("lng", [P, 1], f32, kind="ExternalInput")
    lnb_d = nc.dram_tensor("lnb", [P, 1], f32, kind="ExternalInput")
    w1_d = nc.dram_tensor("w1", [P, 2 * P], f32, kind="ExternalInput")
    b1_d = nc.dram_tensor("b1t", [2 * P, 1], f32, kind="ExternalInput")
    w2_d = nc.dram_tensor("w2", [2 * P, P], f32, kind="ExternalInput")
    b2_d = nc.dram_tensor("b2t", [P, 1], f32, kind="ExternalInput")
    b1e_d = nc.dram_tensor("b1e", [2 * P, 1], f32, kind="ExternalInput")
    y_out = nc.dram_tensor("y_local", [BLOC, C, L], f16, kind="ExternalOutput")

    x_ap = x_in.ap()
    y_ap = y_out.ap()

    from contextlib import contextmanager

    with tile.TileContext(nc) as tc, ExitStack() as ctx:
        # Priority hoist for the per-engine scheduler heaps (lower = earlier).
        # Only the stats chains and the seam-critical last-path sums jump the
        # queue (negative band, sequence preserves emission order); everything
        # else keeps natural emission order. Without the hoist, pending sums
        # interleave into the stats chain on DVE and stretch it ~15 us; with a
        # global sums>multiplies band (tried), e0's multiplies lose to ALL of
        # e1's sums and the store stream starts ~10 us late — both slower.
        BAND_STATS = -100000
        band_seq = {}

        _band_stack = []

        def push_band(b):
            _band_stack.append(tc.cur_priority)
            tc.cur_priority = b + band_seq.get(b, 0)

        def pop_band(b):
            band_seq[b] = tc.cur_priority - b
            tc.cur_priority = _band_stack.pop()

        @contextmanager
        def band(b):
            push_band(b)
            try:
                yield
            finally:
                pop_band(b)
        consts = ctx.enter_context(tc.tile_pool(name="consts", bufs=1))
        xp = ctx.enter_context(tc.tile_pool(name="xp", bufs=XBUFS))
        sm = ctx.enter_context(tc.tile_pool(name="sm", bufs=2))
        pp = ctx.enter_context(tc.tile_pool(name="pp", bufs=1, space="PSUM"))

        def cload(dram, shape):
            # consts ride the gpsimd SWDGE queue so both HWDGE rings (sync =
            # loads, ACT = e1 stores) start their real work immediately
            t = consts.tile(shape, f32, name=dram.name + "_sb", tag=dram.name)
            nc.gpsimd.dma_start(out=t[:], in_=dram.ap()[:, :])
            return t

        a9 = cload(a9_d, [P, 9])
        cb8 = cload(cb_d, [P, 1])
        lng = cload(lng_d, [P, 1])
        lnb = cload(lnb_d, [P, 1])
        w1 = cload(w1_d, [P, 2 * P])
        b1t = cload(b1_d, [2 * P, 1])
        w2 = cload(w2_d, [2 * P, P])
        b2t = cload(b2_d, [P, 1])
        b1e = cload(b1e_d, [2 * P, 1])
        ident = consts.tile([128, 128], f32)
        masks.make_identity(nc, ident[:])
        ones18 = consts.tile([1, P], f32)
        nc.vector.memset(ones18[:], 1.0)
        eps1 = consts.tile([1, 1], f32)
        nc.vector.memset(eps1[:], LN_EPS)

        H = L // 2
        for e in range(BLOC):
            # ---- stream in + per-path sum over l ----
            # One full-row 1 MiB tile per path [128, 4096] fp16 (8 KiB rows,
            # same DMA descriptor shape as the old f32 half tiles but half as
            # many instructions/semaphores). All loads on the load-only sync
            # ring; one pool slot per tile so loads never wait on recycling.
            # Sums alternate ACT / DVE per path. ACT paths (0,2,4,6): one
            # full-tile Copy with f32 accumulator (half-splits cost ~0.7 us
            # fixed overhead per extra ACT instruction — not worth it). DVE
            # paths (1,3,5,7): TWO half-tile reduce_sums into separate
            # columns, folded together at stats time — a 2.2 us reduce caps
            # how long the (priority-hoisted) stats chain can be blocked
            # behind an in-flight DVE op, vs 4.4 us for full-tile reduces.
            # Path 7's second half goes to ACT so both engines finish the
            # example's sums at about the same time.
            xts = []
            ysumT = sm.tile([128, P + 4], f32, tag="ysumT")
            for p in range(P):
                csl = slice(p * 128, (p + 1) * 128)
                h = xp.tile([128, L], f16, tag="x", name=f"x_{e}_{p}")
                if p % 2 == 0:
                    nc.sync.dma_start(out=h[:], in_=x_ap[e, csl, 0:L])
                    nc.scalar.activation(
                        out=h[:], in_=h[:], func=AF.Copy,
                        accum_out=ysumT[:, p:p + 1])
                else:
                    nc.sync.dma_start(out=h[:, 0:H], in_=x_ap[e, csl, 0:H])
                    nc.sync.dma_start(out=h[:, H:L], in_=x_ap[e, csl, H:L])
                    hb = band(BAND_STATS) if p == P - 1 else None
                    if hb:
                        hb.__enter__()
                    nc.vector.reduce_sum(ysumT[:, p:p + 1], h[:, 0:H],
                                         axis=AX.X)
                    if p < P - 1:
                        nc.vector.reduce_sum(
                            ysumT[:, P + p // 2:P + p // 2 + 1], h[:, H:L],
                            axis=AX.X)
                    else:
                        nc.scalar.activation(
                            out=h[:, H:L], in_=h[:, H:L], func=AF.Copy,
                            accum_out=ysumT[:, P + p // 2:P + p // 2 + 1])
                    if hb:
                        hb.__exit__(None, None, None)
                xts.append(h)

            push_band(BAND_STATS)
            # fold the DVE second-half columns back into the odd path sums
            nc.vector.tensor_add(ysumT[:, 1:P:2], ysumT[:, 1:P:2],
                                 ysumT[:, P:P + 4])

            # ---- to [p, d] layout via PE ----
            ysum_ps = pp.tile([P, D], f32, tag="ysum_ps", bufs=2)
            nc.tensor.transpose(ysum_ps[:], ysumT[:, 0:P], ident[:])

            # ---- combined 9-tap grouped conv along d (zero-padded) ----
            ypad = sm.tile([P, D + 8], f32, tag="ypad")
            nc.vector.memset(ypad[:, 0:4], 0.0)
            nc.vector.memset(ypad[:, D + 4:D + 8], 0.0)
            nc.vector.tensor_copy(ypad[:, 4:D + 4], ysum_ps[:])
            acc = [sm.tile([P, D], f32, tag=f"acc{i}", name=f"acc{i}_{e}")
                   for i in range(2)]
            nc.vector.tensor_scalar_mul(acc[0][:], ypad[:, 0:D], a9[:, 0:1])
            cur = 0
            for k in range(1, 9):
                nxt = 1 - cur
                nc.vector.scalar_tensor_tensor(
                    out=acc[nxt][:], in0=ypad[:, k:k + D], scalar=a9[:, k:k + 1],
                    in1=acc[cur][:], op0=OP.mult, op1=OP.add)
                cur = nxt

            # ---- attn = sigmoid(logit + combine_b); crosssum = sum_d attn ----
            attn = sm.tile([P, D], f32, tag="attn")
            rhs2 = sm.tile([P, 2], f32, tag="rhs2")  # [ones | crosssum]
            nc.vector.memset(rhs2[:, 0:1], 1.0)
            nc.scalar.activation(out=attn[:], in_=acc[cur][:], func=AF.Sigmoid,
                                 bias=cb8[:], accum_out=rhs2[:, 1:2])

            # ---- LayerNorm over the 8 paths (crosssum units; 1/D folded) ----
            stats_ps = pp.tile([1, 2], f32, tag="stats")  # [sum, sumsq]
            nc.tensor.matmul(stats_ps[:], rhs2[:, 1:2], rhs2[:], start=True, stop=True)
            musig = sm.tile([1, 2], f32, tag="musig")     # [mu_s, rstd]
            nc.vector.tensor_scalar_mul(musig[:, 0:1], stats_ps[:, 0:1], 1.0 / P)
            musq = sm.tile([1, 1], f32, tag="musq")
            nc.vector.tensor_mul(musq[:], musig[:, 0:1], musig[:, 0:1])
            var_s = sm.tile([1, 1], f32, tag="var_s")
            nc.vector.scalar_tensor_tensor(
                out=var_s[:], in0=stats_ps[:, 1:2], scalar=1.0 / P, in1=musq[:],
                op0=OP.mult, op1=OP.subtract)
            den2 = sm.tile([1, 1], f32, tag="den2")
            nc.vector.tensor_scalar(
                out=den2[:], in0=var_s[:], scalar1=1.0 / (D * D), scalar2=LN_EPS,
                op0=OP.mult, op1=OP.add)
            if RSQRT_POW:
                # rstd = den2^-0.5 in one DVE op (no ACT table swap)
                nc.vector.tensor_scalar(
                    out=musig[:, 1:2], in0=den2[:], scalar1=-0.5, scalar2=None,
                    op0=OP.pow)
            else:
                denom = sm.tile([1, 1], f32, tag="denom")
                nc.scalar.sqrt(denom[:], den2[:])
                nc.vector.reciprocal(musig[:, 1:2], denom[:])
            bc_ps = pp.tile([P, 2], f32, tag="bc")        # broadcast mu/rstd to 8 rows
            nc.tensor.matmul(bc_ps[:], ones18[:], musig[:], start=True, stop=True)
            t8 = sm.tile([P, 1], f32, tag="t8")
            nc.vector.scalar_tensor_tensor(
                out=t8[:], in0=rhs2[:, 1:2], scalar=bc_ps[:, 0:1], in1=bc_ps[:, 1:2],
                op0=OP.subtract, op1=OP.mult)
            h8 = sm.tile([P, 1], f32, tag="h8")
            nc.vector.scalar_tensor_tensor(
                out=h8[:], in0=t8[:], scalar=lng[:], in1=lnb[:],
                op0=OP.mult, op1=OP.add)

            # ---- gate MLP: sigmoid(W2.T gelu(W1.T h + b1) + b2) ----
            # exact erf-gelu: 0.5*(z+b1)*(1+erf((z+b1)/sqrt(2))) — Erf lives in
            # the same ACT table set as Sigmoid, so no table swaps.
            z1_ps = pp.tile([2 * P, 1], f32, tag="z1")
            nc.tensor.matmul(z1_ps[:], w1[:], h8[:], start=True, stop=True)
            e16 = sm.tile([2 * P, 1], f32, tag="e16")
            nc.scalar.activation(out=e16[:], in_=z1_ps[:], func=AF.Erf,
                                 scale=0.7071067811865476, bias=b1e[:])
            z1b = sm.tile([2 * P, 1], f32, tag="z1b")
            nc.vector.tensor_scalar_add(z1b[:], z1_ps[:], b1t[:])
            e1p = sm.tile([2 * P, 1], f32, tag="e1p")
            nc.vector.tensor_scalar_add(e1p[:], e16[:], 1.0)
            h1t = sm.tile([2 * P, 1], f32, tag="h1t")
            nc.vector.scalar_tensor_tensor(
                out=h1t[:], in0=z1b[:], scalar=0.5, in1=e1p[:],
                op0=OP.mult, op1=OP.mult)
            z2_ps = pp.tile([P, 1], f32, tag="z2")
            nc.tensor.matmul(z2_ps[:], w2[:], h1t[:], start=True, stop=True)
            gatet = sm.tile([P, 1], f32, tag="gatet")
            nc.scalar.activation(out=gatet[:], in_=z2_ps[:], func=AF.Sigmoid,
                                 bias=b2t[:])

            # ---- scale = attn * gate, transposed to [d, p] ----
            scale8 = sm.tile([P, D], f32, tag="scale8")
            nc.vector.tensor_scalar_mul(scale8[:], attn[:], gatet[:])
            scaleT_ps = pp.tile([128, P], f32, tag="scaleT", bufs=2)
            nc.tensor.transpose(scaleT_ps[:], scale8[:], ident[0:P, 0:P])
            # scalar operand of tensor_scalar must be f32 (ISA rule); scalar
            # operands are exempt from the DVE 2-byte perf-mode dtype check
            scaleT = sm.tile([128, P], f32, tag="scaleT_sb")
            nc.vector.tensor_copy(scaleT[:], scaleT_ps[:])
            pop_band(BAND_STATS)

            # ---- apply and stream out ----
            # Scaling on DVE (fp16 2x mode). Ring assignment by EXAMPLE:
            # e0 stores ride the sync ring — its 16 load triggers are all
            # issued early, so the ring is drained right when e0's multiplies
            # finish, and the ACT engine stream stays free of store triggers
            # until e1's stats are done (they were delaying e1's stats by
            # ~14 us when everything shared the ACT ring). e1 stores ride the
            # ACT ring, which by then only ran sums + the two stats chains.
            for p in range(P):
                h = xts[p]
                csl = slice(p * 128, (p + 1) * 128)
                sc = scaleT[:, p:p + 1]
                nc.vector.tensor_scalar_mul(h[:], h[:], sc)
                seng = nc.scalar if e == BLOC - 1 else nc.sync
                seng.dma_start(out=y_ap[e, csl, 0:L], in_=h[:])

    nc.compile()
    return nc


def _get_nc():
    global _cached
    if _cached is None:
        _cached = _build()
    return _cached


def _fold_weights(conv1_w, conv2_w, combine_w, combine_b, ln_g, ln_b, W1, b1, W2, b2):
    a9 = np.zeros((P, 9), np.float32)
    a9[:, 2:7] += combine_w[0] * conv1_w
    a9[:, :] += combine_w[1] * conv2_w
    a9 /= L  # fold mean over l into the conv taps
    return {
        "a9": np.ascontiguousarray(a9),
        "cb8": np.full((P, 1), float(combine_b), np.float32),
        "lng": np.ascontiguousarray((ln_g / D).reshape(P, 1).astype(np.float32)),
        "lnb": np.ascontiguousarray(ln_b.reshape(P, 1).astype(np.float32)),
        "w1": np.ascontiguousarray(W1.astype(np.float32)),
        "b1t": np.ascontiguousarray(b1.reshape(2 * P, 1).astype(np.float32)),
        "w2": np.ascontiguousarray(W2.astype(np.float32)),
        "b2t": np.ascontiguousarray(b2.reshape(P, 1).astype(np.float32)),
        "b1e": np.ascontiguousarray(
            (b1 / np.sqrt(2.0)).reshape(2 * P, 1).astype(np.float32)),
    }


def run(x, consts, trace=False, **trace_kwargs):
    from concourse.bass_utils import run_bass_kernel_spmd

    nc = _get_nc()
    core_ids = list(range(N_CORES))
    x16 = x.astype(np.float16) if x.dtype != np.float16 else x
    in_maps = []
    for i in core_ids:
        m = {"x_local": np.ascontiguousarray(x16[i * BLOC:(i + 1) * BLOC])}
        m.update(consts)
        in_maps.append(m)
    try:
        res = run_bass_kernel_spmd(nc, in_maps, core_ids, trace=trace,
                                   **trace_kwargs)
    except Exception:
        # transient NRT_EXEC_UNIT_UNRECOVERABLE after recompiles — one retry
        res = run_bass_kernel_spmd(nc, in_maps, core_ids, trace=trace,
                                   **trace_kwargs)
    out = np.concatenate(
        [res.results[i]["y_local"] for i in core_ids], axis=0
    ).astype(np.float32)
    return out, res


def kernel(x, conv1_w, conv2_w, combine_w, combine_b, ln_g, ln_b, W1, b1, W2, b2):
    x = np.asarray(x, np.float32)
    assert x.shape == (B, C, L), x.shape
    consts = _fold_weights(
        np.asarray(conv1_w, np.float32), np.asarray(conv2_w, np.float32),
        np.asarray(combine_w, np.float32), np.asarray(combine_b, np.float32),
        np.asarray(ln_g, np.float32), np.asarray(ln_b, np.float32),
        np.asarray(W1, np.float32), np.asarray(b1, np.float32),
        np.asarray(W2, np.float32), np.asarray(b2, np.float32))
    out, _ = run(x, consts)
    return out



# revision 30
# speedup vs baseline: 1.1736x; 1.1736x over previous
"""Trainium2 Bass kernel for EnhancedPathAwareECA.

Data-parallel over batch: 16 examples split as 2 per NeuronCore across 8 cores
(no collectives — per-example stats are local). Each core streams its slice of
x through SBUF exactly once: load -> per-path sum over l -> tiny
attention/LN/MLP chain -> in-place channel scaling -> store.

fp16 I/O: x is downcast to fp16 on the host before upload and the output is
stored fp16 and upcast on the host — halves HBM traffic vs f32 (the f32
schedule was already at the chip HBM roofline, ~197 us). All pooled sums
accumulate in f32 (ACT accum_out / DVE reduce output dtype), the stats chain
is f32, and only the streamed tiles + final multiply are fp16 (x quantization
2^-11 -> rel err ~3e-4, far under the 2e-2 gate).

Schedule notes (hard-won on HW):
- Loads own the sync HWDGE ring exclusively; stores ride the scalar (ACT)
  HWDGE ring. HWDGE rings are FIFO: a store waiting on its scale multiply
  would head-of-line-block every later load if they shared a ring. The last
  example's h0 stores switch to the then-idle sync ring (dual-ring drain).
- Each 2 MiB path tile is two independent 1 MiB half-tiles in a 24-slot pool:
  8 spare slots let the next example prefetch through the stats seam.
- Per-path sums: h0 on ACT (activation Copy + accum_out), h1 on DVE
  (reduce_sum) — both are 1x-rate ops, and splitting engines keeps DVE from
  backlogging so the seam-critical last reduce dequeues immediately.
- Scale multiplies on DVE: fp32 tensor_scalar is 2x-rate (2x_2P mode);
  ACT Copy is ~1.7x slower and would pace the drain.
- Stats chain avoids ACT table swaps where possible: everything stays in the
  'sigmoid_and_others' set (sigmoid + erf-based exact gelu + square via DVE);
  only Sqrt forces 2 swaps/example (DVE pow is rejected by walrus, no DVE
  sqrt exists), mostly hidden under concurrent PE/DVE chain ops.
- All weight-only folds (combined 9-tap conv kernel = combine_w-mixed conv1/
  conv2 taps with the 1/l mean fold, 1/D LayerNorm fold into ln_g, b1/sqrt(2)
  for the erf gelu) are precomputed on the host.
"""

import sys
from contextlib import ExitStack

import numpy as np

sys.path.insert(0, "/opt/trn_rl_repo")

N_CORES = 8
B, C, L = 16, 1024, 4096
P, D = 8, 128            # paths, dims per path (C = P*D)
BLOC = B // N_CORES      # examples per core
LN_EPS = 1e-5
XBUFS = 16               # 1 MiB fp16 full-row tile slots (16 MiB SBUF) — the
                         # whole 2-example core slice fits; loads never stall
                         # on pool recycling
RSQRT_POW = False        # DVE pow is rejected by walrus (tensor_scalar_valid_ops)
USE_TTR = False          # tensor_tensor_reduce sums: NRT_EXEC_UNIT_UNRECOVERABLE
                         # status_code=101 on HW (in0==in1==out aliasing?)

_cached = None


def _build():
    import concourse.tile as tile
    from concourse import bacc, masks, mybir

    f32 = mybir.dt.float32
    f16 = mybir.dt.float16
    AX = mybir.AxisListType
    OP = mybir.AluOpType
    AF = mybir.ActivationFunctionType

    nc = bacc.Bacc(
        "TRN2",
        target_bir_lowering=False,
        debug=False,
        num_devices=N_CORES,
    )

    x_in = nc.dram_tensor("x_local", [BLOC, C, L], f16, kind="ExternalInput")
    a9_d = nc.dram_tensor("a9", [P, 9], f32, kind="ExternalInput")
    cb_d = nc.dram_tensor("cb8", [P, 1], f32, kind="ExternalInput")
    lng_d = nc.dram_tensor("lng", [P, 1], f32, kind="ExternalInput")
    lnb_d = nc.dram_tensor("lnb", [P, 1], f32, kind="ExternalInput")
    w1_d = nc.dram_tensor("w1", [P, 2 * P], f32, kind="ExternalInput")
    b1_d = nc.dram_tensor("b1t", [2 * P, 1], f32, kind="ExternalInput")
    w2_d = nc.dram_tensor("w2", [2 * P, P], f32, kind="ExternalInput")
    b2_d = nc.dram_tensor("b2t", [P, 1], f32, kind="ExternalInput")
    b1e_d = nc.dram_tensor("b1e", [2 * P, 1], f32, kind="ExternalInput")
    y_out = nc.dram_tensor("y_local", [BLOC, C, L], f16, kind="ExternalOutput")

    x_ap = x_in.ap()
    y_ap = y_out.ap()

    from contextlib import contextmanager

    with tile.TileContext(nc) as tc, ExitStack() as ctx:
        # Priority-band experiments all regressed on HW (v4 116 us / v5 115 /
        # v6 119 vs 102 with natural emission order) — both a global
        # sums>multiplies band and a negative-priority hoist of just the
        # stats chains disturb the scheduler/autobuf ordering more than they
        # help. Bands are kept as NO-OPS to document the attempts.
        BAND_STATS = 0

        def push_band(b):
            pass

        def pop_band(b):
            pass

        @contextmanager
        def band(b):
            yield
        consts = ctx.enter_context(tc.tile_pool(name="consts", bufs=1))
        xp = ctx.enter_context(tc.tile_pool(name="xp", bufs=XBUFS))
        sm = ctx.enter_context(tc.tile_pool(name="sm", bufs=2))
        pp = ctx.enter_context(tc.tile_pool(name="pp", bufs=1, space="PSUM"))

        def cload(dram, shape):
            # consts ride the gpsimd SWDGE queue so both HWDGE rings (sync =
            # loads, ACT = e1 stores) start their real work immediately
            t = consts.tile(shape, f32, name=dram.name + "_sb", tag=dram.name)
            nc.gpsimd.dma_start(out=t[:], in_=dram.ap()[:, :])
            return t

        a9 = cload(a9_d, [P, 9])
        cb8 = cload(cb_d, [P, 1])
        lng = cload(lng_d, [P, 1])
        lnb = cload(lnb_d, [P, 1])
        w1 = cload(w1_d, [P, 2 * P])
        b1t = cload(b1_d, [2 * P, 1])
        w2 = cload(w2_d, [2 * P, P])
        b2t = cload(b2_d, [P, 1])
        b1e = cload(b1e_d, [2 * P, 1])
        ident = consts.tile([128, 128], f32)
        masks.make_identity(nc, ident[:])
        ones18 = consts.tile([1, P], f32)
        nc.vector.memset(ones18[:], 1.0)
        eps1 = consts.tile([1, 1], f32)
        nc.vector.memset(eps1[:], LN_EPS)

        H = L // 2
        for e in range(BLOC):
            # ---- stream in + per-path sum over l ----
            # One full-row 1 MiB tile per path [128, 4096] fp16 (8 KiB rows,
            # same DMA descriptor shape as the old f32 half tiles but half as
            # many instructions/semaphores). All loads on the load-only sync
            # ring; one pool slot per tile so loads never wait on recycling.
            # Sums alternate ACT (Copy w/ f32 accumulator) / DVE (reduce_sum,
            # ~1.08 ns/elt for fp16 on HW). The last path is loaded and
            # summed as two halves split across both engines so the
            # stats-seam lag after the final load is ~2 us instead of ~4.4.
            xts = []
            ysumT = sm.tile([128, P + 1], f32, tag="ysumT")
            for p in range(P):
                csl = slice(p * 128, (p + 1) * 128)
                h = xp.tile([128, L], f16, tag="x", name=f"x_{e}_{p}")
                if p < P - 1:
                    nc.sync.dma_start(out=h[:], in_=x_ap[e, csl, 0:L])
                    if p % 2 == 0:
                        nc.scalar.activation(
                            out=h[:], in_=h[:], func=AF.Copy,
                            accum_out=ysumT[:, p:p + 1])
                    else:
                        nc.vector.reduce_sum(ysumT[:, p:p + 1], h[:],
                                             axis=AX.X)
                else:
                    nc.sync.dma_start(out=h[:, 0:H], in_=x_ap[e, csl, 0:H])
                    nc.sync.dma_start(out=h[:, H:L], in_=x_ap[e, csl, H:L])
                    nc.vector.reduce_sum(ysumT[:, p:p + 1], h[:, 0:H],
                                         axis=AX.X)
                    nc.scalar.activation(
                        out=h[:, H:L], in_=h[:, H:L], func=AF.Copy,
                        accum_out=ysumT[:, P:P + 1])
                xts.append(h)

            push_band(BAND_STATS)
            nc.vector.tensor_add(ysumT[:, P - 1:P], ysumT[:, P - 1:P],
                                 ysumT[:, P:P + 1])

            # ---- to [p, d] layout via PE ----
            ysum_ps = pp.tile([P, D], f32, tag="ysum_ps", bufs=2)
            nc.tensor.transpose(ysum_ps[:], ysumT[:, 0:P], ident[:])

            # ---- combined 9-tap grouped conv along d (zero-padded) ----
            ypad = sm.tile([P, D + 8], f32, tag="ypad")
            nc.vector.memset(ypad[:, 0:4], 0.0)
            nc.vector.memset(ypad[:, D + 4:D + 8], 0.0)
            nc.vector.tensor_copy(ypad[:, 4:D + 4], ysum_ps[:])
            acc = [sm.tile([P, D], f32, tag=f"acc{i}", name=f"acc{i}_{e}")
                   for i in range(2)]
            nc.vector.tensor_scalar_mul(acc[0][:], ypad[:, 0:D], a9[:, 0:1])
            cur = 0
            for k in range(1, 9):
                nxt = 1 - cur
                nc.vector.scalar_tensor_tensor(
                    out=acc[nxt][:], in0=ypad[:, k:k + D], scalar=a9[:, k:k + 1],
                    in1=acc[cur][:], op0=OP.mult, op1=OP.add)
                cur = nxt

            # ---- attn = sigmoid(logit + combine_b); crosssum = sum_d attn ----
            attn = sm.tile([P, D], f32, tag="attn")
            rhs2 = sm.tile([P, 2], f32, tag="rhs2")  # [ones | crosssum]
            nc.vector.memset(rhs2[:, 0:1], 1.0)
            nc.scalar.activation(out=attn[:], in_=acc[cur][:], func=AF.Sigmoid,
                                 bias=cb8[:], accum_out=rhs2[:, 1:2])

            # ---- LayerNorm over the 8 paths (crosssum units; 1/D folded) ----
            stats_ps = pp.tile([1, 2], f32, tag="stats")  # [sum, sumsq]
            nc.tensor.matmul(stats_ps[:], rhs2[:, 1:2], rhs2[:], start=True, stop=True)
            musig = sm.tile([1, 2], f32, tag="musig")     # [mu_s, rstd]
            nc.vector.tensor_scalar_mul(musig[:, 0:1], stats_ps[:, 0:1], 1.0 / P)
            musq = sm.tile([1, 1], f32, tag="musq")
            nc.vector.tensor_mul(musq[:], musig[:, 0:1], musig[:, 0:1])
            var_s = sm.tile([1, 1], f32, tag="var_s")
            nc.vector.scalar_tensor_tensor(
                out=var_s[:], in0=stats_ps[:, 1:2], scalar=1.0 / P, in1=musq[:],
                op0=OP.mult, op1=OP.subtract)
            den2 = sm.tile([1, 1], f32, tag="den2")
            nc.vector.tensor_scalar(
                out=den2[:], in0=var_s[:], scalar1=1.0 / (D * D), scalar2=LN_EPS,
                op0=OP.mult, op1=OP.add)
            if RSQRT_POW:
                # rstd = den2^-0.5 in one DVE op (no ACT table swap)
                nc.vector.tensor_scalar(
                    out=musig[:, 1:2], in0=den2[:], scalar1=-0.5, scalar2=None,
                    op0=OP.pow)
            else:
                denom = sm.tile([1, 1], f32, tag="denom")
                nc.scalar.sqrt(denom[:], den2[:])
                nc.vector.reciprocal(musig[:, 1:2], denom[:])
            bc_ps = pp.tile([P, 2], f32, tag="bc")        # broadcast mu/rstd to 8 rows
            nc.tensor.matmul(bc_ps[:], ones18[:], musig[:], start=True, stop=True)
            t8 = sm.tile([P, 1], f32, tag="t8")
            nc.vector.scalar_tensor_tensor(
                out=t8[:], in0=rhs2[:, 1:2], scalar=bc_ps[:, 0:1], in1=bc_ps[:, 1:2],
                op0=OP.subtract, op1=OP.mult)
            h8 = sm.tile([P, 1], f32, tag="h8")
            nc.vector.scalar_tensor_tensor(
                out=h8[:], in0=t8[:], scalar=lng[:], in1=lnb[:],
                op0=OP.mult, op1=OP.add)

            # ---- gate MLP: sigmoid(W2.T gelu(W1.T h + b1) + b2) ----
            # exact erf-gelu: 0.5*(z+b1)*(1+erf((z+b1)/sqrt(2))) — Erf lives in
            # the same ACT table set as Sigmoid, so no table swaps.
            z1_ps = pp.tile([2 * P, 1], f32, tag="z1")
            nc.tensor.matmul(z1_ps[:], w1[:], h8[:], start=True, stop=True)
            e16 = sm.tile([2 * P, 1], f32, tag="e16")
            nc.scalar.activation(out=e16[:], in_=z1_ps[:], func=AF.Erf,
                                 scale=0.7071067811865476, bias=b1e[:])
            z1b = sm.tile([2 * P, 1], f32, tag="z1b")
            nc.vector.tensor_scalar_add(z1b[:], z1_ps[:], b1t[:])
            e1p = sm.tile([2 * P, 1], f32, tag="e1p")
            nc.vector.tensor_scalar_add(e1p[:], e16[:], 1.0)
            h1t = sm.tile([2 * P, 1], f32, tag="h1t")
            nc.vector.scalar_tensor_tensor(
                out=h1t[:], in0=z1b[:], scalar=0.5, in1=e1p[:],
                op0=OP.mult, op1=OP.mult)
            z2_ps = pp.tile([P, 1], f32, tag="z2")
            nc.tensor.matmul(z2_ps[:], w2[:], h1t[:], start=True, stop=True)
            gatet = sm.tile([P, 1], f32, tag="gatet")
            nc.scalar.activation(out=gatet[:], in_=z2_ps[:], func=AF.Sigmoid,
                                 bias=b2t[:])

            # ---- scale = attn * gate, transposed to [d, p] ----
            scale8 = sm.tile([P, D], f32, tag="scale8")
            nc.vector.tensor_scalar_mul(scale8[:], attn[:], gatet[:])
            scaleT_ps = pp.tile([128, P], f32, tag="scaleT", bufs=2)
            nc.tensor.transpose(scaleT_ps[:], scale8[:], ident[0:P, 0:P])
            # scalar operand of tensor_scalar must be f32 (ISA rule); scalar
            # operands are exempt from the DVE 2-byte perf-mode dtype check
            scaleT = sm.tile([128, P], f32, tag="scaleT_sb")
            nc.vector.tensor_copy(scaleT[:], scaleT_ps[:])
            pop_band(BAND_STATS)

            # ---- apply and stream out ----
            # Scaling on DVE (fp16 2x mode). Ring assignment by EXAMPLE:
            # e0 stores ride the sync ring — its 16 load triggers are all
            # issued early, so the ring is drained right when e0's multiplies
            # finish, and the ACT engine stream stays free of store triggers
            # until e1's stats are done (they were delaying e1's stats by
            # ~14 us when everything shared the ACT ring). e1 stores ride the
            # ACT ring, which by then only ran sums + the two stats chains.
            for p in range(P):
                h = xts[p]
                csl = slice(p * 128, (p + 1) * 128)
                sc = scaleT[:, p:p + 1]
                nc.vector.tensor_scalar_mul(h[:], h[:], sc)
                seng = nc.scalar if e == BLOC - 1 else nc.sync
                seng.dma_start(out=y_ap[e, csl, 0:L], in_=h[:])

    nc.compile()
    return nc


def _get_nc():
    global _cached
    if _cached is None:
        _cached = _build()
    return _cached


def _fold_weights(conv1_w, conv2_w, combine_w, combine_b, ln_g, ln_b, W1, b1, W2, b2):
    a9 = np.zeros((P, 9), np.float32)
    a9[:, 2:7] += combine_w[0] * conv1_w
    a9[:, :] += combine_w[1] * conv2_w
    a9 /= L  # fold mean over l into the conv taps
    return {
        "a9": np.ascontiguousarray(a9),
        "cb8": np.full((P, 1), float(combine_b), np.float32),
        "lng": np.ascontiguousarray((ln_g / D).reshape(P, 1).astype(np.float32)),
        "lnb": np.ascontiguousarray(ln_b.reshape(P, 1).astype(np.float32)),
        "w1": np.ascontiguousarray(W1.astype(np.float32)),
        "b1t": np.ascontiguousarray(b1.reshape(2 * P, 1).astype(np.float32)),
        "w2": np.ascontiguousarray(W2.astype(np.float32)),
        "b2t": np.ascontiguousarray(b2.reshape(P, 1).astype(np.float32)),
        "b1e": np.ascontiguousarray(
            (b1 / np.sqrt(2.0)).reshape(2 * P, 1).astype(np.float32)),
    }


def run(x, consts, trace=False, **trace_kwargs):
    from concourse.bass_utils import run_bass_kernel_spmd

    nc = _get_nc()
    core_ids = list(range(N_CORES))
    x16 = x.astype(np.float16) if x.dtype != np.float16 else x
    in_maps = []
    for i in core_ids:
        m = {"x_local": np.ascontiguousarray(x16[i * BLOC:(i + 1) * BLOC])}
        m.update(consts)
        in_maps.append(m)
    try:
        res = run_bass_kernel_spmd(nc, in_maps, core_ids, trace=trace,
                                   **trace_kwargs)
    except Exception:
        # transient NRT_EXEC_UNIT_UNRECOVERABLE after recompiles — one retry
        res = run_bass_kernel_spmd(nc, in_maps, core_ids, trace=trace,
                                   **trace_kwargs)
    out = np.concatenate(
        [res.results[i]["y_local"] for i in core_ids], axis=0
    ).astype(np.float32)
    return out, res


def kernel(x, conv1_w, conv2_w, combine_w, combine_b, ln_g, ln_b, W1, b1, W2, b2):
    x = np.asarray(x, np.float32)
    assert x.shape == (B, C, L), x.shape
    consts = _fold_weights(
        np.asarray(conv1_w, np.float32), np.asarray(conv2_w, np.float32),
        np.asarray(combine_w, np.float32), np.asarray(combine_b, np.float32),
        np.asarray(ln_g, np.float32), np.asarray(ln_b, np.float32),
        np.asarray(W1, np.float32), np.asarray(b1, np.float32),
        np.asarray(W2, np.float32), np.asarray(b2, np.float32))
    out, _ = run(x, consts)
    return out

